# revision 6
# baseline (speedup 1.0000x reference)
"""Trainium2 Bass kernel for nn_Decoder (Tacotron2-style attention decoder).

Strategy (8 NeuronCores, one chip):
  - LSTM hidden dims 8-way model-parallel: core j owns hidden slice
    [128j:128(j+1)) of both LSTMs => 512 gate rows per LSTM per core,
    all weights resident in SBUF.
  - Attention batch-sharded: core j owns batches [4j:4j+4) for the
    location conv / energies / softmax / context reduction.
  - Per step, 2 AllGathers: AG1 = [ahT(t) | dhT(t-1)] (hidden slices),
    AG2 = ctx_local (batch slices). 501 steps fully unrolled.
  - Mel/gate projection deferred: h=(dh,ctx) history stored in HBM, one
    batched matmul at the end + ReduceScatter over partial sums.
"""
import numpy as np
from contextlib import ExitStack

import concourse.bass as bass
import concourse.tile as tile
from concourse import bacc, mybir

F32 = mybir.dt.float32
AF = mybir.ActivationFunctionType
OP = mybir.AluOpType

B, T_ENC, T_DEC, D_ENC, N_MEL, KCV = 32, 512, 500, 512, 160, 31
NCORE = 8
HL = 128          # hidden slice per core
GL = 4 * HL       # gate slice per core (i|f|g|o)
BL = 4            # batch slice per core (attention shard)
NT = T_DEC + 1    # 501 decode steps
KC = 69           # conv lhsT contraction: 62 windows + 2 pad + 4 query + 1 bias
NBT = BL * T_ENC  # 2048 (b,t) columns per core

_prog_cache: dict = {}


def _mm(nc, out, lhsT, rhs, start, stop):
    nc.tensor.matmul(out, lhsT, rhs, start=start, stop=stop)


def build_program(n_steps: int = NT):
    if n_steps in _prog_cache:
        return _prog_cache[n_steps]
    nc = bacc.Bacc("TRN2", target_bir_lowering=False, debug=False,
                   num_devices=NCORE)

    def din(name, shape):
        return nc.dram_tensor(name, list(shape), F32, kind="ExternalInput").ap()

    # ---- per-core external inputs ----
    whhA_in = din("whhA_T", (1024, GL))
    wihActx_in = din("wihA_ctxT", (512, GL))
    wihDctx_in = din("wihD_ctxT", (512, GL))
    wihDah_in = din("wihD_ahT", (1024, GL))
    whhD_in = din("whhD_T", (1024, GL))
    wqT_in = din("wqT", (1024, 128))
    preA_in = din("preA", (n_steps, B, GL))
    bD_in = din("bD_row", (1, GL))
    convlhsT_in = din("convlhsT_static", (KC, 128))
    rhs_static_in = din("rhs_static", (7, NBT))      # rows 62..68
    wa4_in = din("wa4", (128, 16))                   # [128,4,4] flattened
    keysT_in = din("mem_keysT", (128, NBT))
    enc_in = din("enc_bd", (NBT, D_ENC))
    maskpen_in = din("maskpen", (BL, T_ENC))
    ident_in = din("ident", (32, 32))
    ones1_in = din("ones1", (1, B))
    selT_in = din("selT", (B, BL))
    wmgT_in = din("wmgT", (HL, 161))
    wmgctxT_in = din("wmgctxT", (512, 161))          # zero except own ctx block

    # ---- per-core external outputs ----
    attn_out = nc.dram_tensor("attn", [n_steps, BL, T_ENC], F32,
                              kind="ExternalOutput").ap()
    nrow_mel = n_steps * B
    rs_rows = nrow_mel // NCORE
    mel_out = nc.dram_tensor("melout", [rs_rows, 161], F32,
                             kind="ExternalOutput").ap()

    # ---- internal dram ----
    aw_dram = nc.dram_tensor("aw_dram", [8, 542], F32).ap()
    dh_hist = nc.dram_tensor("dh_hist", [n_steps, HL, B], F32).ap()
    ctxT_hist = nc.dram_tensor("ctxT_hist", [n_steps, 128, 128], F32).ap()
    mel_part = nc.dram_tensor("mel_part", [nrow_mel, 161], F32).ap()

    # ---- persistent sbuf ----
    sb = lambda name, shape: nc.alloc_sbuf_tensor(name, list(shape), F32)
    whhA_sb = sb("whhA_sb", (128, 8, GL))
    wihActx_sb = sb("wihActx_sb", (128, 4, GL))
    wihDctx_sb = sb("wihDctx_sb", (128, 4, GL))
    wihDah_sb = sb("wihDah_sb", (128, 8, GL))
    whhD_sb = sb("whhD_sb", (128, 8, GL))
    wqT_sb = sb("wqT_sb", (128, 8, 128))
    enc_sb = sb("enc_sb", (128, 16, D_ENC))
    keysT_sb = sb("keysT_sb", (128, NBT))
    convlhsT = sb("convlhsT", (KC, 128))
    rhs_buf = sb("rhs_buf", (KC, NBT))
    wa4_sb = sb("wa4_sb", (128, 4, 4))
    buf3 = sb("buf3", (128, 112))
    id_sb = sb("id_sb", (32, 32))
    bD_sb = sb("bD_sb", (1, GL))
    ones1_sb = sb("ones1_sb", (1, B))
    selT_sb = sb("selT_sb", (B, BL))
    maskpen_sb = sb("maskpen_sb", (BL, T_ENC))
    wmgT_sb = sb("wmgT_sb", (HL, 161))
    wmgctxT_sb = sb("wmgctxT_sb", (128, 4, 161))
    aw_sb = sb("aw_sb", (BL, 544))
    awt_sb = sb("awt_sb", (BL, 544))
    ahT_sb = sb("ahT_sb", (128, 8, B))
    dhT_sb = sb("dhT_sb", (128, 8, B))
    ctxT_sb = sb("ctxT_sb", (128, 4, B))
    payload1 = sb("payload1", (128, 2 * B))
    cA_sb = sb("cA_sb", (B, HL))
    cD_sb = sb("cD_sb", (B, HL))

    with tile.TileContext(nc) as tc, ExitStack() as ctx:
        sp = ctx.enter_context(tc.tile_pool(name="sp", bufs=2))
        tp_pool = ctx.enter_context(tc.tile_pool(name="tanhp", bufs=2))
        dram = ctx.enter_context(tc.tile_pool(name="dram", bufs=3, space="DRAM"))
        ps_gates = ctx.enter_context(tc.tile_pool(name="ps_gates", bufs=2, space="PSUM"))
        ps_ep = ctx.enter_context(tc.tile_pool(name="ps_ep", bufs=2, space="PSUM"))
        ps_e4 = ctx.enter_context(tc.tile_pool(name="ps_e4", bufs=1, space="PSUM"))
        ps_ctx = ctx.enter_context(tc.tile_pool(name="ps_ctx", bufs=1, space="PSUM"))
        ps_small = ctx.enter_context(tc.tile_pool(name="ps_small", bufs=2, space="PSUM"))

        # ================= prologue =================
        def load_tiled(dst, src, n_kk, width):
            # dst [128, n_kk, width] <- src [(n_kk*128), width]
            for kk in range(n_kk):
                nc.sync.dma_start(
                    dst.ap()[:, kk, :],
                    bass.AP(src.tensor, kk * 128 * width, [[width, 128], [1, width]]))

        load_tiled(whhA_sb, whhA_in, 8, GL)
        load_tiled(wihActx_sb, wihActx_in, 4, GL)
        load_tiled(wihDctx_sb, wihDctx_in, 4, GL)
        load_tiled(wihDah_sb, wihDah_in, 8, GL)
        load_tiled(whhD_sb, whhD_in, 8, GL)
        load_tiled(wqT_sb, wqT_in, 8, 128)
        load_tiled(enc_sb, enc_in, 16, D_ENC)
        load_tiled(wmgctxT_sb, wmgctxT_in, 4, 161)
        nc.sync.dma_start(keysT_sb.ap(), keysT_in)
        nc.sync.dma_start(convlhsT.ap(), convlhsT_in)
        nc.sync.dma_start(rhs_buf.ap()[62:69, :], rhs_static_in)
        nc.sync.dma_start(wa4_sb.ap().rearrange("p a b -> p (a b)"), wa4_in)
        nc.sync.dma_start(id_sb.ap(), ident_in)
        nc.sync.dma_start(bD_sb.ap(), bD_in)
        nc.sync.dma_start(ones1_sb.ap(), ones1_in)
        nc.sync.dma_start(selT_sb.ap(), selT_in)
        nc.sync.dma_start(maskpen_sb.ap(), maskpen_in)
        nc.sync.dma_start(wmgT_sb.ap(), wmgT_in)

        for t_ in (aw_sb, awt_sb, cA_sb, cD_sb, payload1, buf3):
            nc.vector.memset(t_.ap(), 0.0)
        for t_ in (ahT_sb, dhT_sb, ctxT_sb):
            nc.vector.memset(t_.ap().rearrange("p a b -> p (a b)"), 0.0)
        nc.sync.dma_start(aw_dram[0:4, :], aw_sb.ap()[:, 0:542])
        nc.sync.dma_start(aw_dram[4:8, :], awt_sb.ap()[:, 0:542])

        rg = [list(range(NCORE))]

        def lstm_pointwise(gps, c_sb, payload_col):
            """gates psum [B, GL] (i|f|g|o) -> h_localT into payload1 col."""
            sig_if = sp.tile([B, 2 * HL], F32, tag="sigif")
            nc.scalar.activation(sig_if[:], gps[:, 0:2 * HL], AF.Sigmoid)
            tng = sp.tile([B, HL], F32, tag="tng")
            nc.scalar.activation(tng[:], gps[:, 2 * HL:3 * HL], AF.Tanh)
            sgo = sp.tile([B, HL], F32, tag="sgo")
            nc.scalar.activation(sgo[:], gps[:, 3 * HL:4 * HL], AF.Sigmoid)
            m1 = sp.tile([B, HL], F32, tag="m1")
            nc.vector.tensor_mul(m1[:], sig_if[:, 0:HL], tng[:])
            m2 = sp.tile([B, HL], F32, tag="m2")
            nc.vector.tensor_mul(m2[:], sig_if[:, HL:2 * HL], c_sb.ap())
            nc.vector.tensor_add(c_sb.ap(), m1[:], m2[:])
            tc_ = sp.tile([B, HL], F32, tag="tc")
            nc.scalar.activation(tc_[:], c_sb.ap(), AF.Tanh)
            hl_ = sp.tile([B, HL], F32, tag="hl")
            nc.vector.tensor_mul(hl_[:], sgo[:], tc_[:])
            pT = ps_small.tile([128, B], F32, tag="pssm")
            nc.tensor.transpose(pT[:], hl_[:], id_sb.ap())
            nc.vector.tensor_copy(payload1.ap()[:, payload_col:payload_col + B], pT[:])

        # ================= decode steps =================
        for t in range(n_steps):
            # ---- attention-LSTM gates ----
            gA = ps_gates.tile([B, GL], F32, tag="gates")
            for kk in range(8):
                _mm(nc, gA[:], ahT_sb.ap()[:, kk, :], whhA_sb.ap()[:, kk, :],
                    kk == 0, False)
            for kk in range(4):
                _mm(nc, gA[:], ctxT_sb.ap()[:, kk, :], wihActx_sb.ap()[:, kk, :],
                    False, kk == 3)
            preA_t = sp.tile([B, GL], F32, tag="preA")
            nc.sync.dma_start(preA_t[:], preA_in[t])
            nc.vector.tensor_add(gA[:], gA[:], preA_t[:])
            lstm_pointwise(gA, cA_sb, 0)

            # ---- AG1: [ahT(t) | dhT(t-1)] ----
            c1i = dram.tile([128, 2 * B], F32, tag="c1i")
            nc.sync.dma_start(c1i[:], payload1.ap())
            c1o = dram.tile([128 * NCORE, 2 * B], F32, tag="c1o")
            nc.gpsimd.collective_compute(
                "AllGather", OP.bypass, replica_groups=rg,
                ins=[c1i.opt()], outs=[c1o.opt()])
            nc.sync.dma_start(
                ahT_sb.ap(),
                bass.AP(c1o.tensor, 0, [[2 * B, 128], [128 * 2 * B, 8], [1, B]]))
            nc.sync.dma_start(
                dhT_sb.ap(),
                bass.AP(c1o.tensor, B, [[2 * B, 128], [128 * 2 * B, 8], [1, B]]))

            # ---- query (full, then select local 4 rows) ----
            q32 = ps_small.tile([B, 128], F32, tag="pssm")
            for kk in range(8):
                _mm(nc, q32[:], ahT_sb.ap()[:, kk, :], wqT_sb.ap()[:, kk, :],
                    kk == 0, kk == 7)
            q32s = sp.tile([B, 128], F32, tag="q32s")
            nc.scalar.copy(q32s[:], q32[:])
            q4 = ps_small.tile([BL, 128], F32, tag="pssm")
            _mm(nc, q4[:], selT_sb.ap(), q32s[:], True, True)
            nc.scalar.copy(convlhsT.ap()[64:68, :], q4[:])

            # ---- im2col (DRAM -> rhs_buf rows 0..61) ----
            for i in range(2):
                for g in range(4):
                    k0, nk = g * 8, (8 if g < 3 else 7)
                    nc.sync.dma_start(
                        bass.AP(rhs_buf, (i * KCV + k0) * NBT,
                                [[NBT, nk], [T_ENC, BL], [1, T_ENC]]),
                        bass.AP(aw_dram.tensor, i * 4 * 542 + k0,
                                [[1, nk], [542, BL], [1, T_ENC]]))

            # ---- conv+query+bias matmul, +keys, tanh ----
            tanhT = tp_pool.tile([128, NBT], F32, tag="tanhT")
            for c in range(4):
                ep = ps_ep.tile([128, T_ENC], F32, tag="ep")
                _mm(nc, ep[:], convlhsT.ap(),
                    rhs_buf.ap()[:, c * T_ENC:(c + 1) * T_ENC], True, True)
                nc.vector.tensor_add(
                    ep[:], ep[:], keysT_sb.ap()[:, c * T_ENC:(c + 1) * T_ENC])
                nc.scalar.activation(
                    tanhT[:, c * T_ENC:(c + 1) * T_ENC], ep[:], AF.Tanh)

            # ---- energies e[b,t] via diag-lhsT, exp, softmax pieces ----
            e4 = ps_e4.tile([BL, T_ENC], F32, tag="e4")
            for c in range(4):
                _mm(nc, e4[:], wa4_sb.ap()[:, c, :],
                    tanhT[:, c * T_ENC:(c + 1) * T_ENC], c == 0, c == 3)
            nc.vector.tensor_add(e4[:], e4[:], maskpen_sb.ap())
            aww = sp.tile([BL, T_ENC], F32, tag="aww")
            s4 = sp.tile([BL, 1], F32, tag="s4")
            nc.scalar.activation(aww[:], e4[:], AF.Exp, accum_out=s4[:])
            rs4 = sp.tile([BL, 1], F32, tag="rs4")
            nc.vector.reciprocal(rs4[:], s4[:])

            # ---- transpose aw-hat into diag-padded blockdiag lhsT ----
            for ch in range(4):
                tpp = ps_small.tile([128, BL], F32, tag="pssm")
                nc.tensor.transpose(tpp[:], aww[:, ch * 128:(ch + 1) * 128],
                                    id_sb.ap()[0:4, 0:4])
                nc.vector.tensor_copy(
                    bass.AP(buf3, ch * 7 + 3, [[112, 128], [28, BL]]), tpp[:])

            # ---- ctx = (aw-hat @ enc) * (1/s) ----
            cps = ps_ctx.tile([BL, D_ENC], F32, tag="cps")
            for kk in range(16):
                b = kk // 4
                _mm(nc, cps[:],
                    bass.AP(buf3, kk * 7 + 3 - b, [[112, 128], [1, BL]]),
                    enc_sb.ap()[:, kk, :], kk == 0, kk == 15)
            ctxl = sp.tile([BL, D_ENC], F32, tag="ctxl")
            nc.scalar.activation(ctxl[:], cps[:], AF.Copy, bias=0.0, scale=rs4[:])

            # ---- aw out + window update ----
            awo = sp.tile([BL, T_ENC], F32, tag="awo")
            nc.vector.tensor_scalar_mul(awo[:], aww[:], rs4[:])
            nc.sync.dma_start(attn_out[t], awo[:])
            nc.vector.tensor_copy(aw_sb.ap()[:, 15:527], awo[:])
            nc.vector.tensor_add(awt_sb.ap()[:, 15:527], awt_sb.ap()[:, 15:527],
                                 awo[:])
            nc.sync.dma_start(aw_dram[0:4, :], aw_sb.ap()[:, 0:542])
            nc.sync.dma_start(aw_dram[4:8, :], awt_sb.ap()[:, 0:542])

            # ---- AG2: ctx batch-slices ----
            c2i = dram.tile([BL, D_ENC], F32, tag="c2i")
            nc.sync.dma_start(c2i[:], ctxl[:])
            c2o = dram.tile([B, D_ENC], F32, tag="c2o")
            nc.gpsimd.collective_compute(
                "AllGather", OP.bypass, replica_groups=rg,
                ins=[c2i.opt()], outs=[c2o.opt()])
            for kk in range(4):
                nc.sync.dma_start(
                    ctxT_sb.ap()[:, kk, :],
                    bass.AP(c2o.tensor, kk * 128, [[1, 128], [D_ENC, B]]))
            nc.sync.dma_start(ctxT_hist[t],
                              ctxT_sb.ap().rearrange("p a b -> p (a b)"))

            # ---- decoder-LSTM gates ----
            gD = ps_gates.tile([B, GL], F32, tag="gates")
            for kk in range(8):
                _mm(nc, gD[:], dhT_sb.ap()[:, kk, :], whhD_sb.ap()[:, kk, :],
                    kk == 0, False)
            for kk in range(8):
                _mm(nc, gD[:], ahT_sb.ap()[:, kk, :], wihDah_sb.ap()[:, kk, :],
                    False, False)
            for kk in range(4):
                _mm(nc, gD[:], ctxT_sb.ap()[:, kk, :], wihDctx_sb.ap()[:, kk, :],
                    False, False)
            _mm(nc, gD[:], ones1_sb.ap(), bD_sb.ap(), False, True)
            lstm_pointwise(gD, cD_sb, B)
            nc.sync.dma_start(dh_hist[t], payload1.ap()[:, B:2 * B])

        # ================= mel/gate projection =================
        n_chunk = (n_steps + 15) // 16
        for c in range(n_chunk):
            tn = min(16, n_steps - c * 16)
            ncol = tn * B
            rdh = sp.tile([HL, 512], F32, tag="rdh")
            nc.sync.dma_start(
                rdh[:, 0:ncol],
                bass.AP(dh_hist.tensor, c * 16 * HL * B,
                        [[B, HL], [HL * B, tn], [1, B]]))
            rctx_all = tp_pool.tile([128, 4, 512], F32, tag="tanhT",
                                    name=f"rctx_{c}")
            rctx = [rctx_all[:, kk, :] for kk in range(4)]
            for kk in range(4):
                nc.sync.dma_start(
                    rctx[kk][:, 0:ncol],
                    bass.AP(ctxT_hist.tensor, c * 16 * 128 * 128 + kk * 32,
                            [[128, 128], [128 * 128, tn], [1, B]]))
            for half, (m0, m1) in enumerate(((0, 81), (81, 161))):
                mw = m1 - m0
                pm = ps_gates.tile([mw, 512], F32, tag="gates")
                _mm(nc, pm[:, 0:ncol], wmgT_sb.ap()[:, m0:m1], rdh[:, 0:ncol],
                    True, False)
                for kk in range(4):
                    _mm(nc, pm[:, 0:ncol], wmgctxT_sb.ap()[:, kk, m0:m1],
                        rctx[kk][:, 0:ncol], False, kk == 3)
                ev = sp.tile([mw, 512], F32, tag="mev")
                nc.scalar.copy(ev[:, 0:ncol], pm[:, 0:ncol])
                nc.sync.dma_start(
                    bass.AP(mel_part.tensor, c * 512 * 161 + m0,
                            [[1, mw], [161, ncol]]),
                    ev[:, 0:ncol])

        mel_rs = dram.tile([rs_rows, 161], F32, tag="melrs")
        nc.gpsimd.collective_compute(
            "ReduceScatter", OP.add, replica_groups=rg,
            ins=[bass.AP(mel_part.tensor, 0, [[161, nrow_mel], [1, 161]])],
            outs=[mel_rs.opt()])
        nc.sync.dma_start(mel_out[:], mel_rs[:])

    nc.compile()
    _prog_cache[n_steps] = nc
    return nc


# ======================= host side =======================

def host_inputs(inputs: dict, n_steps: int = NT):
    f = lambda x: np.asarray(x, dtype=np.float32)
    enc = f(inputs["encoder_out"]); mels = f(inputs["mels"])
    mask = np.asarray(inputs["mask"])
    w_pre1, b_pre1 = f(inputs["w_pre1"]), f(inputs["b_pre1"])
    w_pre2, b_pre2 = f(inputs["w_pre2"]), f(inputs["b_pre2"])
    wih_a, whh_a = f(inputs["wih_a"]), f(inputs["whh_a"])
    bih_a, bhh_a = f(inputs["bih_a"]), f(inputs["bhh_a"])
    wq, bq = f(inputs["wq"]), f(inputs["bq"])
    wk, bk = f(inputs["wk"]), f(inputs["bk"])
    conv_w, conv_b = f(inputs["conv_w"]), f(inputs["conv_b"])
    wl, bl = f(inputs["wl"]), f(inputs["bl"])
    wa, ba = f(inputs["wa"]), f(inputs["ba"])
    wih_d, whh_d = f(inputs["wih_d"]), f(inputs["whh_d"])
    bih_d, bhh_d = f(inputs["bih_d"]), f(inputs["bhh_d"])
    wm, bm = f(inputs["wm"]), f(inputs["bm"])
    wg, bg = f(inputs["wg"]), f(inputs["bg"])

    # prenet over all frames
    mel_seq = np.concatenate([np.zeros((B, 1, N_MEL), np.float32), mels], 1)
    pre = np.maximum(mel_seq @ w_pre1.T + b_pre1, 0.0)
    pre = np.maximum(pre @ w_pre2.T + b_pre2, 0.0)      # [B, NT, 256]
    pre = np.ascontiguousarray(pre.transpose(1, 0, 2))[:n_steps]  # [nt, B, 256]

    # location-conv stack folded through wl (+ all e-biases)
    w_stack = np.einsum("dc,cik->ikd", wl, conv_w).reshape(62, 128)
    bias_row = wl @ conv_b + bl + bq                    # [128]

    mask1 = mask[:, 0, :].astype(np.float32)            # [B, T_ENC]

    wmg = np.vstack([wm, wg])                           # [161, 1536]

    in_maps = []
    for j in range(NCORE):
        gsel = np.concatenate([blk * 1024 + j * HL + np.arange(HL)
                               for blk in range(4)])
        bsel = np.arange(BL * j, BL * (j + 1))

        preA = pre.reshape(-1, 256) @ wih_a[gsel, :256].T
        preA += (bih_a + bhh_a)[gsel]
        preA = preA.reshape(n_steps, B, GL)

        convlhsT_static = np.zeros((KC, 128), np.float32)
        convlhsT_static[0:62] = w_stack
        convlhsT_static[68] = bias_row

        rhs_static = np.zeros((7, NBT), np.float32)
        for m in range(BL):
            rhs_static[2 + m, m * T_ENC:(m + 1) * T_ENC] = 1.0
        rhs_static[6, :] = 1.0

        wa4 = np.zeros((128, 4, 4), np.float32)
        for c in range(4):
            wa4[:, c, c] = wa[0]

        keys = enc[bsel] @ wk.T + bk                    # [BL, T_ENC, 128]
        keysT = np.ascontiguousarray(
            keys.transpose(2, 0, 1).reshape(128, NBT))

        selT = np.zeros((B, BL), np.float32)
        selT[bsel, np.arange(BL)] = 1.0

        wmgT = np.ascontiguousarray(wmg[:, j * HL:(j + 1) * HL].T)  # [HL,161]
        wmgctxT = np.zeros((512, 161), np.float32)
        if j < 4:
            wmgctxT[j * 128:(j + 1) * 128, :] = wmg[:, 1024 + j * 128:
                                                    1024 + (j + 1) * 128].T

        in_maps.append(dict(
            whhA_T=np.ascontiguousarray(whh_a[gsel].T),
            wihA_ctxT=np.ascontiguousarray(wih_a[gsel, 256:768].T),
            wihD_ctxT=np.ascontiguousarray(wih_d[gsel, :512].T),
            wihD_ahT=np.ascontiguousarray(wih_d[gsel, 512:1536].T),
            whhD_T=np.ascontiguousarray(whh_d[gsel].T),
            wqT=np.ascontiguousarray(wq.T),
            preA=np.ascontiguousarray(preA),
            bD_row=np.ascontiguousarray((bih_d + bhh_d)[gsel][None, :]),
            convlhsT_static=convlhsT_static,
            rhs_static=rhs_static,
            wa4=wa4.reshape(128, 16),
            mem_keysT=keysT,
            enc_bd=np.ascontiguousarray(enc[bsel].reshape(NBT, D_ENC)),
            maskpen=np.ascontiguousarray(-1e30 * mask1[bsel] + ba[0]),
            ident=np.eye(32, dtype=np.float32),
            ones1=np.ones((1, B), np.float32),
            selT=selT,
            wmgT=wmgT,
            wmgctxT=wmgctxT,
        ))
    # bias of mel/gate folded at assembly time (bm, bg added on host)
    return in_maps, (bm, bg)


def assemble(results, biases, n_steps: int = NT):
    bm, bg = biases
    attn = np.zeros((n_steps, B, T_ENC), np.float32)
    for j in range(NCORE):
        attn[:, BL * j:BL * (j + 1), :] = results[j]["attn"]
    mel_flat = np.concatenate([results[j]["melout"] for j in range(NCORE)], 0)
    mel_flat = mel_flat.reshape(n_steps, B, 161)
    mels_out = mel_flat[:, :, :160] + bm
    gates_out = mel_flat[:, :, 160:161] + bg
    return mels_out, gates_out, attn


def kernel(**inputs):
    from concourse.bass_utils import run_bass_kernel_spmd
    nc = build_program(NT)
    in_maps, biases = host_inputs(inputs, NT)
    res = run_bass_kernel_spmd(nc, in_maps, list(range(NCORE)))
    return assemble([res.results[j] for j in range(NCORE)], biases, NT)


# revision 11
# speedup vs baseline: 1.1818x; 1.1818x over previous
"""Trainium2 Bass kernel for nn_Decoder (Tacotron2-style attention decoder).

Strategy (8 NeuronCores, one chip):
  - LSTM hidden dims 8-way model-parallel: core j owns hidden slice
    [128j:128(j+1)) of both LSTMs => 512 gate rows per LSTM per core,
    all weights resident in SBUF.
  - Attention batch-sharded: core j owns batches [4j:4j+4) for the
    location conv / energies / softmax / context reduction.
  - Per step, 2 AllGathers: AG1 = [ahT(t) | dhT(t-1)] (hidden slices),
    AG2 = ctx_local (batch slices). 501 steps fully unrolled.
  - Mel/gate projection deferred: h=(dh,ctx) history stored in HBM, one
    batched matmul at the end + ReduceScatter over partial sums.
"""
import numpy as np
from contextlib import ExitStack

import concourse.bass as bass
import concourse.tile as tile
from concourse import bacc, mybir

F32 = mybir.dt.float32
AF = mybir.ActivationFunctionType
OP = mybir.AluOpType

B, T_ENC, T_DEC, D_ENC, N_MEL, KCV = 32, 512, 500, 512, 160, 31
NCORE = 8
HL = 128          # hidden slice per core
GL = 4 * HL       # gate slice per core (i|f|g|o)
BL = 4            # batch slice per core (attention shard)
NT = T_DEC + 1    # 501 decode steps
KC = 69           # conv lhsT contraction: 62 windows + 2 pad + 4 query + 1 bias
NBT = BL * T_ENC  # 2048 (b,t) columns per core

_prog_cache: dict = {}


_no_mm = [False]
def _mm(nc, out, lhsT, rhs, start, stop):
    if _no_mm[0] and not (start and stop):
        if start:
            nc.tensor.matmul(out, lhsT, rhs, start=True, stop=True)
        return
    nc.tensor.matmul(out, lhsT, rhs, start=start, stop=stop)


def build_program(n_steps: int = NT, ablate: str = ""):
    key = (n_steps, ablate)
    if key in _prog_cache:
        return _prog_cache[key]
    no_cc = "cc" in ablate
    no_gather = "gather" in ablate
    no_mm = "mm" in ablate
    no_hist = "hist" in ablate
    no_im2col = "im2col" in ablate
    no_prea = "prea" in ablate
    nc = bacc.Bacc("TRN2", target_bir_lowering=False, debug=False,
                   num_devices=NCORE)

    def din(name, shape):
        return nc.dram_tensor(name, list(shape), F32, kind="ExternalInput").ap()

    # ---- per-core external inputs ----
    whhA_in = din("whhA_T", (1024, GL))
    wihActx_in = din("wihA_ctxT", (512, GL))
    wihDctx_in = din("wihD_ctxT", (512, GL))
    wihDah_in = din("wihD_ahT", (1024, GL))
    whhD_in = din("whhD_T", (1024, GL))
    wqT_in = din("wqT", (1024, 128))
    preA_in = din("preA", (n_steps, B, GL))
    bD_in = din("bD_row", (1, GL))
    convlhsT_in = din("convlhsT_static", (KC, 128))
    rhs_static_in = din("rhs_static", (7, NBT))      # rows 62..68
    wa4_in = din("wa4", (128, 16))                   # [128,4,4] flattened
    keysT_in = din("mem_keysT", (128, NBT))
    enc_in = din("enc_bd", (NBT, D_ENC))
    maskpen_in = din("maskpen", (BL, T_ENC))
    ident_in = din("ident", (32, 32))
    ones1_in = din("ones1", (1, B))
    selT_in = din("selT", (B, BL))
    wmgT_in = din("wmgT", (HL, 161))
    wmgctxT_in = din("wmgctxT", (512, 161))          # zero except own ctx block

    # ---- per-core external outputs ----
    attn_out = nc.dram_tensor("attn", [n_steps, BL, T_ENC], F32,
                              kind="ExternalOutput").ap()
    nrow_mel = n_steps * B
    rs_rows = nrow_mel // NCORE
    mel_out = nc.dram_tensor("melout", [rs_rows, 161], F32,
                             kind="ExternalOutput").ap()

    # ---- internal dram ----
    aw_dram = nc.dram_tensor("aw_dram", [8, 542], F32).ap()
    dh_hist = nc.dram_tensor("dh_hist", [n_steps, HL, B], F32).ap()
    ctxT_hist = nc.dram_tensor("ctxT_hist", [n_steps, 128, 128], F32).ap()
    mel_part = nc.dram_tensor("mel_part", [nrow_mel, 161], F32).ap()

    # ---- persistent sbuf ----
    sb = lambda name, shape: nc.alloc_sbuf_tensor(name, list(shape), F32)
    whhA_sb = sb("whhA_sb", (128, 8, GL))
    wihActx_sb = sb("wihActx_sb", (128, 4, GL))
    wihDctx_sb = sb("wihDctx_sb", (128, 4, GL))
    wihDah_sb = sb("wihDah_sb", (128, 8, GL))
    whhD_sb = sb("whhD_sb", (128, 8, GL))
    wqT_sb = sb("wqT_sb", (128, 8, 128))
    enc_sb = sb("enc_sb", (128, 16, D_ENC))
    keysT_sb = sb("keysT_sb", (128, NBT))
    convlhsT = sb("convlhsT", (KC, 128))
    rhs_buf = sb("rhs_buf", (KC, NBT))
    wa4_sb = sb("wa4_sb", (128, 4, 4))
    buf3 = sb("buf3", (128, 112))
    id_sb = sb("id_sb", (32, 32))
    bD_sb = sb("bD_sb", (1, GL))
    ones1_sb = sb("ones1_sb", (1, B))
    selT_sb = sb("selT_sb", (B, BL))
    maskpen_sb = sb("maskpen_sb", (BL, T_ENC))
    wmgT_sb = sb("wmgT_sb", (HL, 161))
    wmgctxT_sb = sb("wmgctxT_sb", (128, 4, 161))
    aw_sb = sb("aw_sb", (BL, 544))
    awt_sb = sb("awt_sb", (BL, 544))
    ahT_sb = sb("ahT_sb", (128, 8, B))
    dhT_sb = sb("dhT_sb", (128, 8, B))
    ctxT_sb = sb("ctxT_sb", (128, 4, B))
    payload1 = sb("payload1", (128, 2 * B))
    cA_sb = sb("cA_sb", (B, HL))
    cD_sb = sb("cD_sb", (B, HL))

    _no_mm[0] = no_mm
    with tile.TileContext(nc) as tc, ExitStack() as ctx:
        sp = ctx.enter_context(tc.tile_pool(name="sp", bufs=2))
        tp_pool = ctx.enter_context(tc.tile_pool(name="tanhp", bufs=2))
        dram = ctx.enter_context(tc.tile_pool(name="dram", bufs=3, space="DRAM"))
        ps_gates = ctx.enter_context(tc.tile_pool(name="ps_gates", bufs=2, space="PSUM"))
        ps_ep = ctx.enter_context(tc.tile_pool(name="ps_ep", bufs=2, space="PSUM"))
        ps_e4 = ctx.enter_context(tc.tile_pool(name="ps_e4", bufs=1, space="PSUM"))
        ps_ctx = ctx.enter_context(tc.tile_pool(name="ps_ctx", bufs=1, space="PSUM"))
        ps_small = ctx.enter_context(tc.tile_pool(name="ps_small", bufs=2, space="PSUM"))

        # ================= prologue =================
        def load_tiled(dst, src, n_kk, width):
            # dst [128, n_kk, width] <- src [(n_kk*128), width]
            for kk in range(n_kk):
                nc.sync.dma_start(
                    dst.ap()[:, kk, :],
                    bass.AP(src.tensor, kk * 128 * width, [[width, 128], [1, width]]))

        load_tiled(whhA_sb, whhA_in, 8, GL)
        load_tiled(wihActx_sb, wihActx_in, 4, GL)
        load_tiled(wihDctx_sb, wihDctx_in, 4, GL)
        load_tiled(wihDah_sb, wihDah_in, 8, GL)
        load_tiled(whhD_sb, whhD_in, 8, GL)
        load_tiled(wqT_sb, wqT_in, 8, 128)
        load_tiled(enc_sb, enc_in, 16, D_ENC)
        load_tiled(wmgctxT_sb, wmgctxT_in, 4, 161)
        nc.sync.dma_start(keysT_sb.ap(), keysT_in)
        nc.sync.dma_start(convlhsT.ap(), convlhsT_in)
        nc.sync.dma_start(rhs_buf.ap()[62:69, :], rhs_static_in)
        nc.sync.dma_start(wa4_sb.ap().rearrange("p a b -> p (a b)"), wa4_in)
        nc.sync.dma_start(id_sb.ap(), ident_in)
        nc.sync.dma_start(bD_sb.ap(), bD_in)
        nc.sync.dma_start(ones1_sb.ap(), ones1_in)
        nc.sync.dma_start(selT_sb.ap(), selT_in)
        nc.sync.dma_start(maskpen_sb.ap(), maskpen_in)
        nc.sync.dma_start(wmgT_sb.ap(), wmgT_in)

        for t_ in (aw_sb, awt_sb, cA_sb, cD_sb, payload1, buf3):
            nc.vector.memset(t_.ap(), 0.0)
        for t_ in (ahT_sb, dhT_sb, ctxT_sb):
            nc.vector.memset(t_.ap().rearrange("p a b -> p (a b)"), 0.0)
        nc.sync.dma_start(aw_dram[0:4, :], aw_sb.ap()[:, 0:542])
        nc.sync.dma_start(aw_dram[4:8, :], awt_sb.ap()[:, 0:542])

        rg = [list(range(NCORE))]

        def lstm_pointwise(gps, c_sb, payload_col):
            """gates psum [B, GL] (i|f|g|o) -> h_localT into payload1 col."""
            sig_if = sp.tile([B, 2 * HL], F32, tag="sigif")
            nc.scalar.activation(sig_if[:], gps[:, 0:2 * HL], AF.Sigmoid)
            tng = sp.tile([B, HL], F32, tag="tng")
            nc.scalar.activation(tng[:], gps[:, 2 * HL:3 * HL], AF.Tanh)
            sgo = sp.tile([B, HL], F32, tag="sgo")
            nc.scalar.activation(sgo[:], gps[:, 3 * HL:4 * HL], AF.Sigmoid)
            m1 = sp.tile([B, HL], F32, tag="m1")
            nc.vector.tensor_mul(m1[:], sig_if[:, 0:HL], tng[:])
            m2 = sp.tile([B, HL], F32, tag="m2")
            nc.vector.tensor_mul(m2[:], sig_if[:, HL:2 * HL], c_sb.ap())
            nc.vector.tensor_add(c_sb.ap(), m1[:], m2[:])
            tc_ = sp.tile([B, HL], F32, tag="tc")
            nc.scalar.activation(tc_[:], c_sb.ap(), AF.Tanh)
            hl_ = sp.tile([B, HL], F32, tag="hl")
            nc.vector.tensor_mul(hl_[:], sgo[:], tc_[:])
            pT = ps_small.tile([128, B], F32, tag="pssm")
            nc.tensor.transpose(pT[:], hl_[:], id_sb.ap())
            nc.vector.tensor_copy(payload1.ap()[:, payload_col:payload_col + B], pT[:])

        # ================= decode steps =================
        for t in range(n_steps):
            # ---- attention-LSTM gates ----
            gA = ps_gates.tile([B, GL], F32, tag="gates")
            for kk in range(8):
                _mm(nc, gA[:], ahT_sb.ap()[:, kk, :], whhA_sb.ap()[:, kk, :],
                    kk == 0, False)
            for kk in range(4):
                _mm(nc, gA[:], ctxT_sb.ap()[:, kk, :], wihActx_sb.ap()[:, kk, :],
                    False, kk == 3)
            if not no_prea:
                preA_t = sp.tile([B, GL], F32, tag="preA")
                nc.sync.dma_start(preA_t[:], preA_in[t])
                nc.vector.tensor_add(gA[:], gA[:], preA_t[:])
            lstm_pointwise(gA, cA_sb, 0)

            # ---- AG1: [ahT(t) | dhT(t-1)] ----
            c1i = dram.tile([128, 2 * B], F32, tag="c1i")
            nc.sync.dma_start(c1i[:], payload1.ap())
            c1o = dram.tile([128 * NCORE, 2 * B], F32, tag="c1o")
            if not no_cc:
                nc.gpsimd.collective_compute(
                    "AllGather", OP.bypass, replica_groups=rg,
                    ins=[c1i.opt()], outs=[c1o.opt()])
            else:
                nc.sync.dma_start(bass.AP(c1o.tensor, 0, [[2*B, 128], [1, 2*B]]), c1i[:])
            if not no_gather:
                nc.sync.dma_start(
                    ahT_sb.ap(),
                    bass.AP(c1o.tensor, 0, [[2 * B, 128], [128 * 2 * B, 8], [1, B]]))
                nc.sync.dma_start(
                    dhT_sb.ap(),
                    bass.AP(c1o.tensor, B, [[2 * B, 128], [128 * 2 * B, 8], [1, B]]))

            # ---- query (full, then select local 4 rows) ----
            q32 = ps_small.tile([B, 128], F32, tag="pssm")
            for kk in range(8):
                _mm(nc, q32[:], ahT_sb.ap()[:, kk, :], wqT_sb.ap()[:, kk, :],
                    kk == 0, kk == 7)
            q32s = sp.tile([B, 128], F32, tag="q32s")
            nc.scalar.copy(q32s[:], q32[:])
            q4 = ps_small.tile([BL, 128], F32, tag="pssm")
            _mm(nc, q4[:], selT_sb.ap(), q32s[:], True, True)
            nc.scalar.copy(convlhsT.ap()[64:68, :], q4[:])

            # ---- im2col (DRAM -> rhs_buf rows 0..61) ----
            if not no_im2col:
                for i in range(2):
                    for g in range(4):
                        k0, nk = g * 8, (8 if g < 3 else 7)
                        nc.sync.dma_start(
                            bass.AP(rhs_buf, (i * KCV + k0) * NBT,
                                    [[NBT, nk], [T_ENC, BL], [1, T_ENC]]),
                            bass.AP(aw_dram.tensor, i * 4 * 542 + k0,
                                    [[1, nk], [542, BL], [1, T_ENC]]))

            # ---- conv+query+bias matmul, +keys, tanh ----
            tanhT = tp_pool.tile([128, NBT], F32, tag="tanhT")
            for c in range(4):
                ep = ps_ep.tile([128, T_ENC], F32, tag="ep")
                _mm(nc, ep[:], convlhsT.ap(),
                    rhs_buf.ap()[:, c * T_ENC:(c + 1) * T_ENC], True, True)
                nc.vector.tensor_add(
                    ep[:], ep[:], keysT_sb.ap()[:, c * T_ENC:(c + 1) * T_ENC])
                nc.scalar.activation(
                    tanhT[:, c * T_ENC:(c + 1) * T_ENC], ep[:], AF.Tanh)

            # ---- energies e[b,t] via diag-lhsT, exp, softmax pieces ----
            e4 = ps_e4.tile([BL, T_ENC], F32, tag="e4")
            for c in range(4):
                _mm(nc, e4[:], wa4_sb.ap()[:, c, :],
                    tanhT[:, c * T_ENC:(c + 1) * T_ENC], c == 0, c == 3)
            nc.vector.tensor_add(e4[:], e4[:], maskpen_sb.ap())
            aww = sp.tile([BL, T_ENC], F32, tag="aww")
            s4 = sp.tile([BL, 1], F32, tag="s4")
            nc.scalar.activation(aww[:], e4[:], AF.Exp, accum_out=s4[:])
            rs4 = sp.tile([BL, 1], F32, tag="rs4")
            nc.vector.reciprocal(rs4[:], s4[:])

            # ---- transpose aw-hat into diag-padded blockdiag lhsT ----
            for ch in range(4):
                tpp = ps_small.tile([128, BL], F32, tag="pssm")
                nc.tensor.transpose(tpp[:], aww[:, ch * 128:(ch + 1) * 128],
                                    id_sb.ap()[0:4, 0:4])
                nc.vector.tensor_copy(
                    bass.AP(buf3, ch * 7 + 3, [[112, 128], [28, BL]]), tpp[:])

            # ---- ctx = (aw-hat @ enc) * (1/s) ----
            cps = ps_ctx.tile([BL, D_ENC], F32, tag="cps")
            for kk in range(16):
                b = kk // 4
                _mm(nc, cps[:],
                    bass.AP(buf3, kk * 7 + 3 - b, [[112, 128], [1, BL]]),
                    enc_sb.ap()[:, kk, :], kk == 0, kk == 15)
            ctxl = sp.tile([BL, D_ENC], F32, tag="ctxl")
            nc.scalar.activation(ctxl[:], cps[:], AF.Copy, bias=0.0, scale=rs4[:])

            # ---- aw out + window update ----
            awo = sp.tile([BL, T_ENC], F32, tag="awo")
            nc.vector.tensor_scalar_mul(awo[:], aww[:], rs4[:])
            if not no_hist:
                nc.sync.dma_start(attn_out[t], awo[:])
            nc.vector.tensor_copy(aw_sb.ap()[:, 15:527], awo[:])
            nc.vector.tensor_add(awt_sb.ap()[:, 15:527], awt_sb.ap()[:, 15:527],
                                 awo[:])
            if not no_im2col:
                nc.sync.dma_start(aw_dram[0:4, :], aw_sb.ap()[:, 0:542])
                nc.sync.dma_start(aw_dram[4:8, :], awt_sb.ap()[:, 0:542])

            # ---- AG2: ctx batch-slices ----
            c2i = dram.tile([BL, D_ENC], F32, tag="c2i")
            nc.sync.dma_start(c2i[:], ctxl[:])
            c2o = dram.tile([B, D_ENC], F32, tag="c2o")
            if not no_cc:
                nc.gpsimd.collective_compute(
                    "AllGather", OP.bypass, replica_groups=rg,
                    ins=[c2i.opt()], outs=[c2o.opt()])
            else:
                nc.sync.dma_start(bass.AP(c2o.tensor, 0, [[D_ENC, BL], [1, D_ENC]]), c2i[:])
            if not no_gather:
                for kk in range(4):
                    nc.sync.dma_start(
                        ctxT_sb.ap()[:, kk, :],
                        bass.AP(c2o.tensor, kk * 128, [[1, 128], [D_ENC, B]]))
            if not no_hist:
                nc.sync.dma_start(ctxT_hist[t],
                                  ctxT_sb.ap().rearrange("p a b -> p (a b)"))

            # ---- decoder-LSTM gates ----
            gD = ps_gates.tile([B, GL], F32, tag="gates")
            for kk in range(8):
                _mm(nc, gD[:], dhT_sb.ap()[:, kk, :], whhD_sb.ap()[:, kk, :],
                    kk == 0, False)
            for kk in range(8):
                _mm(nc, gD[:], ahT_sb.ap()[:, kk, :], wihDah_sb.ap()[:, kk, :],
                    False, False)
            for kk in range(4):
                _mm(nc, gD[:], ctxT_sb.ap()[:, kk, :], wihDctx_sb.ap()[:, kk, :],
                    False, False)
            _mm(nc, gD[:], ones1_sb.ap(), bD_sb.ap(), False, True)
            lstm_pointwise(gD, cD_sb, B)
            if not no_hist:
                nc.sync.dma_start(dh_hist[t], payload1.ap()[:, B:2 * B])

        # ================= mel/gate projection =================
        n_chunk = (n_steps + 15) // 16
        for c in range(n_chunk):
            tn = min(16, n_steps - c * 16)
            ncol = tn * B
            rdh = sp.tile([HL, 512], F32, tag="rdh")
            nc.sync.dma_start(
                rdh[:, 0:ncol],
                bass.AP(dh_hist.tensor, c * 16 * HL * B,
                        [[B, HL], [HL * B, tn], [1, B]]))
            rctx_all = tp_pool.tile([128, 4, 512], F32, tag="tanhT",
                                    name=f"rctx_{c}")
            rctx = [rctx_all[:, kk, :] for kk in range(4)]
            for kk in range(4):
                nc.sync.dma_start(
                    rctx[kk][:, 0:ncol],
                    bass.AP(ctxT_hist.tensor, c * 16 * 128 * 128 + kk * 32,
                            [[128, 128], [128 * 128, tn], [1, B]]))
            for half, (m0, m1) in enumerate(((0, 81), (81, 161))):
                mw = m1 - m0
                pm = ps_gates.tile([mw, 512], F32, tag="gates")
                _mm(nc, pm[:, 0:ncol], wmgT_sb.ap()[:, m0:m1], rdh[:, 0:ncol],
                    True, False)
                for kk in range(4):
                    _mm(nc, pm[:, 0:ncol], wmgctxT_sb.ap()[:, kk, m0:m1],
                        rctx[kk][:, 0:ncol], False, kk == 3)
                ev = sp.tile([mw, 512], F32, tag="mev")
                nc.scalar.copy(ev[:, 0:ncol], pm[:, 0:ncol])
                nc.sync.dma_start(
                    bass.AP(mel_part.tensor, c * 512 * 161 + m0,
                            [[1, mw], [161, ncol]]),
                    ev[:, 0:ncol])

        mel_rs = dram.tile([rs_rows, 161], F32, tag="melrs")
        nc.gpsimd.collective_compute(
            "ReduceScatter", OP.add, replica_groups=rg,
            ins=[bass.AP(mel_part.tensor, 0, [[161, nrow_mel], [1, 161]])],
            outs=[mel_rs.opt()])
        nc.sync.dma_start(mel_out[:], mel_rs[:])

    nc.compile()
    _prog_cache[key] = nc
    return nc


# ======================= host side =======================

def host_inputs(inputs: dict, n_steps: int = NT):
    f = lambda x: np.asarray(x, dtype=np.float32)
    enc = f(inputs["encoder_out"]); mels = f(inputs["mels"])
    mask = np.asarray(inputs["mask"])
    w_pre1, b_pre1 = f(inputs["w_pre1"]), f(inputs["b_pre1"])
    w_pre2, b_pre2 = f(inputs["w_pre2"]), f(inputs["b_pre2"])
    wih_a, whh_a = f(inputs["wih_a"]), f(inputs["whh_a"])
    bih_a, bhh_a = f(inputs["bih_a"]), f(inputs["bhh_a"])
    wq, bq = f(inputs["wq"]), f(inputs["bq"])
    wk, bk = f(inputs["wk"]), f(inputs["bk"])
    conv_w, conv_b = f(inputs["conv_w"]), f(inputs["conv_b"])
    wl, bl = f(inputs["wl"]), f(inputs["bl"])
    wa, ba = f(inputs["wa"]), f(inputs["ba"])
    wih_d, whh_d = f(inputs["wih_d"]), f(inputs["whh_d"])
    bih_d, bhh_d = f(inputs["bih_d"]), f(inputs["bhh_d"])
    wm, bm = f(inputs["wm"]), f(inputs["bm"])
    wg, bg = f(inputs["wg"]), f(inputs["bg"])

    # prenet over all frames
    mel_seq = np.concatenate([np.zeros((B, 1, N_MEL), np.float32), mels], 1)
    pre = np.maximum(mel_seq @ w_pre1.T + b_pre1, 0.0)
    pre = np.maximum(pre @ w_pre2.T + b_pre2, 0.0)      # [B, NT, 256]
    pre = np.ascontiguousarray(pre.transpose(1, 0, 2))[:n_steps]  # [nt, B, 256]

    # location-conv stack folded through wl (+ all e-biases)
    w_stack = np.einsum("dc,cik->ikd", wl, conv_w).reshape(62, 128)
    bias_row = wl @ conv_b + bl + bq                    # [128]

    mask1 = mask[:, 0, :].astype(np.float32)            # [B, T_ENC]

    wmg = np.vstack([wm, wg])                           # [161, 1536]

    in_maps = []
    for j in range(NCORE):
        gsel = np.concatenate([blk * 1024 + j * HL + np.arange(HL)
                               for blk in range(4)])
        bsel = np.arange(BL * j, BL * (j + 1))

        preA = pre.reshape(-1, 256) @ wih_a[gsel, :256].T
        preA += (bih_a + bhh_a)[gsel]
        preA = preA.reshape(n_steps, B, GL)

        convlhsT_static = np.zeros((KC, 128), np.float32)
        convlhsT_static[0:62] = w_stack
        convlhsT_static[68] = bias_row

        rhs_static = np.zeros((7, NBT), np.float32)
        for m in range(BL):
            rhs_static[2 + m, m * T_ENC:(m + 1) * T_ENC] = 1.0
        rhs_static[6, :] = 1.0

        wa4 = np.zeros((128, 4, 4), np.float32)
        for c in range(4):
            wa4[:, c, c] = wa[0]

        keys = enc[bsel] @ wk.T + bk                    # [BL, T_ENC, 128]
        keysT = np.ascontiguousarray(
            keys.transpose(2, 0, 1).reshape(128, NBT))

        selT = np.zeros((B, BL), np.float32)
        selT[bsel, np.arange(BL)] = 1.0

        wmgT = np.ascontiguousarray(wmg[:, j * HL:(j + 1) * HL].T)  # [HL,161]
        wmgctxT = np.zeros((512, 161), np.float32)
        if j < 4:
            wmgctxT[j * 128:(j + 1) * 128, :] = wmg[:, 1024 + j * 128:
                                                    1024 + (j + 1) * 128].T

        in_maps.append(dict(
            whhA_T=np.ascontiguousarray(whh_a[gsel].T),
            wihA_ctxT=np.ascontiguousarray(wih_a[gsel, 256:768].T),
            wihD_ctxT=np.ascontiguousarray(wih_d[gsel, :512].T),
            wihD_ahT=np.ascontiguousarray(wih_d[gsel, 512:1536].T),
            whhD_T=np.ascontiguousarray(whh_d[gsel].T),
            wqT=np.ascontiguousarray(wq.T),
            preA=np.ascontiguousarray(preA),
            bD_row=np.ascontiguousarray((bih_d + bhh_d)[gsel][None, :]),
            convlhsT_static=convlhsT_static,
            rhs_static=rhs_static,
            wa4=wa4.reshape(128, 16),
            mem_keysT=keysT,
            enc_bd=np.ascontiguousarray(enc[bsel].reshape(NBT, D_ENC)),
            maskpen=np.ascontiguousarray(-1e30 * mask1[bsel] + ba[0]),
            ident=np.eye(32, dtype=np.float32),
            ones1=np.ones((1, B), np.float32),
            selT=selT,
            wmgT=wmgT,
            wmgctxT=wmgctxT,
        ))
    # bias of mel/gate folded at assembly time (bm, bg added on host)
    return in_maps, (bm, bg)


def assemble(results, biases, n_steps: int = NT):
    bm, bg = biases
    attn = np.zeros((n_steps, B, T_ENC), np.float32)
    for j in range(NCORE):
        attn[:, BL * j:BL * (j + 1), :] = results[j]["attn"]
    mel_flat = np.concatenate([results[j]["melout"] for j in range(NCORE)], 0)
    mel_flat = mel_flat.reshape(n_steps, B, 161)
    mels_out = mel_flat[:, :, :160] + bm
    gates_out = mel_flat[:, :, 160:161] + bg
    return mels_out, gates_out, attn


def kernel(**inputs):
    from concourse.bass_utils import run_bass_kernel_spmd
    nc = build_program(NT)
    in_maps, biases = host_inputs(inputs, NT)
    res = run_bass_kernel_spmd(nc, in_maps, list(range(NCORE)))
    return assemble([res.results[j] for j in range(NCORE)], biases, NT)
